# revision 15
# baseline (speedup 1.0000x reference)
"""GNN message passing (scatter-add of gathered edge features) on 8 TRN2 cores.

out[n] = sum over edges (s,d) with d==n of x[s].

Sharding: nodes are sorted by in-degree (descending) GLOBALLY and dealt
round-robin to the 8 cores (core = rank % 8, per-core position
m = rank // 8), so every core sees an identical degree profile and the
shared tier structure T is tight. Sorted node m -> supertile m//128,
partition m%128; edge rank r of node m goes to tier r: tier t of a
supertile is one 128-slot block (partition = node) holding each node's
t-th edge, zero-padded where deg <= t. Supertile s needs
T[s] = max(1, maxdeg(s)) tiers; sorting makes T monotone, so within a
group of 32 supertiles every tier covers a prefix of supertiles and one
matmul with the fixed identity stationary (bf16 [128,128],
FWL-eligible) accumulates that tier for all covered supertiles at once
(moving up to 32*32 = 1024 bf16 columns):
psum[128, s*32:(s+1)*32] += xj_block(s, t).

A group's PSUM tile [128, 32*32 f32] spans TWO 2KB banks; matmul pieces
are split at the 16-supertile bank boundary and the first matmul
touching each bank carries start=True (start marks that whole 2KB zero
region pending-zero; a second start would re-mark already-written
columns). PSUM is copied (cast to bf16) to SBUF on ACT and DMA'd out on
the ACT queue; xj loads are split 8-way round-robin across the SP and
ACT HWDGE queues (measured ~1.5x DMA bandwidth vs one queue).

The tier structure T (shared by all cores so the SPMD program is
identical) is data-dependent; kernels are cached per (T, reps).
"""
import sys
import numpy as np

sys.path.insert(0, '/opt/trn_rl_repo')

N = 100000
D = 32
NC = 8
NPC = N // NC            # dst nodes per core
SUP = 128                # nodes per supertile (one PSUM region set)
NSUP = -(-NPC // SUP)    # supertiles per core
NPAD = NSUP * SUP
GRP = 32                 # supertiles per group (psum tile [128, GRP*32])
BANK = 16                # supertiles per PSUM bank (2KB / (32*4B))
DMA_SPLIT = 4            # xj chunks per group, round-robin over queues
DMA_QUEUES = ("sync", "scalar")

_cache = {}


def _groups():
    """List of (sup_start, sup_end) group ranges."""
    return [(a, min(a + GRP, NSUP)) for a in range(0, NSUP, GRP)]


def _plan(T):
    """Column layout + matmul plan for the prefix-fused structure.

    Returns (colmap, plan, nblk):
      colmap[s, t] = global block column (-1 unused)
      plan = per group: (c0, c1, [(t, n_sup, col0)]) where col0 is the
        group-relative block column of tier t's supertile prefix.
    """
    Tmax = max(T)
    colmap = np.full((NSUP, Tmax), -1, np.int64)
    plan = []
    col = 0
    for (sa, sb) in _groups():
        c0 = col
        tiers = []
        tq = [T[sa + s] for s in range(sb - sa)]
        for t in range(max(tq)):
            n = sum(1 for v in tq if v > t)          # prefix length
            assert all(v > t for v in tq[:n]), "T not monotone"
            tiers.append((t, n, col - c0))
            for s in range(n):
                colmap[sa + s, t] = col + s
            col += n
        plan.append((c0, col, tiers))
    return colmap, plan, col


def _build(T, reps):
    """T: tuple of NSUP tier counts (>=1). reps: hardware-loop repetitions
    of the full body (reps>1 is used by the timing harness to amortize
    dispatch overhead)."""
    import concourse.bacc as bacc
    import concourse.tile as tile
    import concourse.mybir as mybir
    from contextlib import nullcontext

    nc = bacc.Bacc("TRN2", target_bir_lowering=False, debug=False,
                   num_devices=NC)
    f32 = mybir.dt.float32
    bf16 = mybir.dt.bfloat16
    _, plan, NBLK = _plan(T)

    xj = nc.dram_tensor("xj", (128, NBLK * D), bf16,
                        kind="ExternalInput").ap()
    s1 = nc.dram_tensor("s1", (128, 128), bf16, kind="ExternalInput").ap()
    y = nc.dram_tensor("y", (128, NSUP * D), bf16, kind="ExternalOutput").ap()

    with tile.TileContext(nc) as tc:
        with (
            tc.tile_pool(name="c", bufs=1) as cpool,
            tc.tile_pool(name="x", bufs=3) as xpool,
            tc.tile_pool(name="st", bufs=2) as spool,
            tc.tile_pool(name="ps", bufs=3, space="PSUM") as ppool,
        ):
            s1_t = cpool.tile([128, 128], bf16)
            nc.sync.dma_start(s1_t[:], s1[:])

            loop = (tc.For_i(0, reps, 1,
                             hint_engines=(mybir.EngineType.PE,),
                             staggered_reset=True)
                    if reps > 1 else nullcontext())
            with loop:
                for gi, (sa, sb) in enumerate(_groups()):
                    c0, c1, tiers = plan[gi]
                    nb = c1 - c0
                    nsup = sb - sa
                    xa = xpool.tile([128, nb * D], bf16)
                    step = -(-nb // DMA_SPLIT)
                    for i in range(DMA_SPLIT):
                        lo, hi = i * step, min(nb, (i + 1) * step)
                        if lo >= hi:
                            continue
                        eng = getattr(nc, DMA_QUEUES[i % len(DMA_QUEUES)])
                        eng.dma_start(xa[:, lo * D:hi * D],
                                      xj[:, (c0 + lo) * D:(c0 + hi) * D])
                    ps = ppool.tile([128, nsup * D], f32)
                    for i, (t, n, col0) in enumerate(tiers):
                        n_next = tiers[i + 1][1] if i + 1 < len(tiers) else 0
                        # piece boundaries: prefix-shrink point (for stop)
                        # and PSUM bank boundaries (matmul must not cross)
                        cuts = {0, n}
                        if 0 < n_next < n:
                            cuts.add(n_next)
                        cuts.update(b for b in range(BANK, n, BANK))
                        cuts = sorted(cuts)
                        for lo, hi in zip(cuts[:-1], cuts[1:]):
                            # the first matmul touching each bank must set
                            # start (start marks that bank's whole 2KB zero
                            # region; a second start would re-mark
                            # already-written columns pending-zero)
                            nc.tensor.matmul(
                                ps[:, lo * D:hi * D],
                                s1_t[:],
                                xa[:, (col0 + lo) * D:(col0 + hi) * D],
                                start=(t == 0 and lo % BANK == 0),
                                stop=(lo >= n_next),
                                skip_group_check=True)
                    st = spool.tile([128, nsup * D], bf16)
                    nc.scalar.copy(st[:], ps[:])
                    nc.scalar.dma_start(y[:, sa * D:sb * D], st[:])

    nc.compile()
    return nc


def _prep_inputs(x, edge_index):
    """Returns (in_maps, T, order). order[rank] = original node id of the
    rank-th node in the global degree sort; rank -> core rank % NC,
    per-core position rank // NC."""
    import ml_dtypes
    x = np.ascontiguousarray(np.asarray(x), dtype=np.float32)
    ei = np.asarray(edge_index)
    src = ei[0].astype(np.int64)
    dst = ei[1].astype(np.int64)
    xpad = np.zeros((N + 1, D), np.float32)
    xpad[:N] = x
    xpad_bf = xpad.astype(ml_dtypes.bfloat16)

    deg = np.bincount(dst, minlength=N)
    order = np.argsort(-deg, kind="stable")            # node ids, deg desc
    rank = np.empty(N, np.int64)
    rank[order] = np.arange(N)
    deg_sorted = deg[order]

    # shared tier profile: supertile s of any core holds global ranks
    # {8*(128s)+k .. } -> max degree = deg at rank 8*128*s (first of sup)
    maxd = deg_sorted[::NC * SUP][:NSUP]
    T_common = np.ones(NSUP, np.int64)
    T_common[:len(maxd)] = np.maximum(1, maxd)
    run = T_common
    T_common = np.maximum.accumulate(run[::-1])[::-1]  # monotone safeguard

    T = tuple(int(v) for v in T_common)
    colmap, plan, NBLK = _plan(T)

    s1 = np.eye(128, dtype=ml_dtypes.bfloat16)

    erank = rank[dst]                                  # global rank per edge
    ecore = erank % NC
    mpos = erank // NC                                 # per-core position
    in_maps = []
    for k in range(NC):
        m = ecore == k
        s_k = src[m]
        mk = mpos[m]
        # rank of each edge within its dst node
        o2 = np.argsort(mk, kind="stable")
        s_k = s_k[o2]
        mk = mk[o2]
        cnt = np.bincount(mk, minlength=NPAD)
        cum = np.zeros(NPAD + 1, np.int64)
        np.cumsum(cnt, out=cum[1:])
        r = np.arange(len(mk)) - cum[mk]
        blk = colmap[mk // SUP, r]                     # block column per edge
        assert (blk >= 0).all()
        p = mk % SUP                                   # partition per edge
        offs = np.full((128, NBLK), N, np.int64)
        offs[p, blk] = s_k
        xjm = xpad_bf[offs.reshape(-1)].reshape(128, NBLK * D)
        in_maps.append({"xj": xjm, "s1": s1})
    return in_maps, T, order


def kernel(x, edge_index):
    from concourse import bass_utils

    in_maps, T, order = _prep_inputs(x, edge_index)
    key = (T, 1)
    if key not in _cache:
        _cache[key] = _build(T, 1)
    nc = _cache[key]

    res = None
    for attempt in range(3):
        try:
            res = bass_utils.run_bass_kernel_spmd(nc, in_maps,
                                                  core_ids=list(range(NC)))
            break
        except Exception:
            if attempt == 2:
                raise
    out = np.empty((N, D), np.float32)
    for k in range(NC):
        y = np.asarray(res.results[k]["y"], np.float32)
        y = y.reshape(128, NSUP, D).transpose(1, 0, 2).reshape(NPAD, D)
        # core k holds global ranks k, k+8, k+16, ... in position order
        ids = order[k::NC]
        out[ids] = y[:len(ids)]
    return out


# revision 16
# speedup vs baseline: 1.1082x; 1.1082x over previous
"""GNN message passing (scatter-add of gathered edge features) on 8 TRN2 cores.

out[n] = sum over edges (s,d) with d==n of x[s].

Sharding: dst nodes split across 8 cores (12500 each). Host sorts each
core's nodes by in-degree (descending) and maps sorted node m to
supertile m//128, partition m%128. Edge rank r of node m goes to tier r:
tier t of a supertile is one 128-slot block (partition = node) holding
each node's t-th edge, padded with zero feature rows where deg <= t.
Supertile s needs T[s] = max(1, maxdeg(s)) tiers; degree sorting makes T
monotone, so within a group of 16 supertiles every tier covers a prefix
of supertiles and ONE matmul with the fixed identity stationary
(bf16 [128,128], FWL-eligible) accumulates that tier for all covered
supertiles at once: psum[128, s*32:(s+1)*32] += xj_block(s,t). Moving
width is up to 16*32=512 bf16 columns. PSUM (f32) is copied (cast to
bf16) to SBUF on ACT and DMA'd out on the ACT queue; xj loads are split
4-way round-robin across the SP and ACT HWDGE queues (measured ~1.5x
DMA bandwidth vs one queue).

The tier structure T (elementwise max across cores so the SPMD program
is identical) is data-dependent; kernels are cached per (T, reps).
"""
import sys
import numpy as np

sys.path.insert(0, '/opt/trn_rl_repo')

N = 100000
D = 32
NC = 8
NPC = N // NC            # dst nodes per core
SUP = 128                # nodes per supertile (one PSUM region)
NSUP = -(-NPC // SUP)    # supertiles per core
NPAD = NSUP * SUP
GRP = 16                 # supertiles per group (psum tile [128, GRP*32])
DMA_SPLIT = 4            # xj chunks per group, round-robin over queues
DMA_QUEUES = ("sync", "scalar")

_cache = {}


def _groups():
    """List of (sup_start, sup_end) group ranges."""
    return [(a, min(a + GRP, NSUP)) for a in range(0, NSUP, GRP)]


def _plan(T):
    """Column layout + matmul plan for the prefix-fused structure.

    Returns (colmap, plan, nblk):
      colmap[s, t] = global block column (-1 unused)
      plan = per group: (c0, c1, [(t, n_sup, col0)]) where col0 is the
        group-relative block column of tier t's supertile prefix.
    """
    Tmax = max(T)
    colmap = np.full((NSUP, Tmax), -1, np.int64)
    plan = []
    col = 0
    for (sa, sb) in _groups():
        c0 = col
        tiers = []
        tq = [T[sa + s] for s in range(sb - sa)]
        for t in range(max(tq)):
            n = sum(1 for v in tq if v > t)          # prefix length
            assert all(v > t for v in tq[:n]), "T not monotone"
            tiers.append((t, n, col - c0))
            for s in range(n):
                colmap[sa + s, t] = col + s
            col += n
        plan.append((c0, col, tiers))
    return colmap, plan, col


def _build(T, reps):
    """T: tuple of NSUP tier counts (>=1). reps: hardware-loop repetitions
    of the full body (reps>1 is used by the timing harness to amortize
    dispatch overhead)."""
    import concourse.bacc as bacc
    import concourse.tile as tile
    import concourse.mybir as mybir
    from contextlib import nullcontext

    nc = bacc.Bacc("TRN2", target_bir_lowering=False, debug=False,
                   num_devices=NC)
    f32 = mybir.dt.float32
    bf16 = mybir.dt.bfloat16
    _, plan, NBLK = _plan(T)

    xj = nc.dram_tensor("xj", (128, NBLK * D), bf16,
                        kind="ExternalInput").ap()
    s1 = nc.dram_tensor("s1", (128, 128), bf16, kind="ExternalInput").ap()
    y = nc.dram_tensor("y", (128, NSUP * D), bf16, kind="ExternalOutput").ap()

    with tile.TileContext(nc) as tc:
        with (
            tc.tile_pool(name="c", bufs=1) as cpool,
            tc.tile_pool(name="x", bufs=3) as xpool,
            tc.tile_pool(name="st", bufs=2) as spool,
            tc.tile_pool(name="ps", bufs=4, space="PSUM") as ppool,
        ):
            s1_t = cpool.tile([128, 128], bf16)
            nc.sync.dma_start(s1_t[:], s1[:])

            loop = (tc.For_i(0, reps, 1,
                             hint_engines=(mybir.EngineType.PE,),
                             staggered_reset=True)
                    if reps > 1 else nullcontext())
            with loop:
                for gi, (sa, sb) in enumerate(_groups()):
                    c0, c1, tiers = plan[gi]
                    nb = c1 - c0
                    nsup = sb - sa
                    xa = xpool.tile([128, nb * D], bf16)
                    step = -(-nb // DMA_SPLIT)
                    for i in range(DMA_SPLIT):
                        lo, hi = i * step, min(nb, (i + 1) * step)
                        if lo >= hi:
                            continue
                        eng = getattr(nc, DMA_QUEUES[i % len(DMA_QUEUES)])
                        eng.dma_start(xa[:, lo * D:hi * D],
                                      xj[:, (c0 + lo) * D:(c0 + hi) * D])
                    ps = ppool.tile([128, nsup * D], f32)
                    for i, (t, n, col0) in enumerate(tiers):
                        n_next = tiers[i + 1][1] if i + 1 < len(tiers) else 0
                        pieces = []
                        if n_next > 0:
                            pieces.append((0, n_next, False))
                        if n > n_next:
                            pieces.append((n_next, n, True))
                        for (lo, hi, stp) in pieces:
                            # only the first matmul of the group's region may
                            # set start: start marks the whole 2KB zero
                            # region, and a second start would re-mark
                            # already-written columns pending-zero
                            nc.tensor.matmul(
                                ps[:, lo * D:hi * D],
                                s1_t[:],
                                xa[:, (col0 + lo) * D:(col0 + hi) * D],
                                start=(t == 0 and lo == 0), stop=stp,
                                skip_group_check=True)
                    st = spool.tile([128, nsup * D], bf16)
                    nc.scalar.copy(st[:], ps[:])
                    nc.scalar.dma_start(y[:, sa * D:sb * D], st[:])

    nc.compile()
    return nc


def _prep_inputs(x, edge_index):
    """Returns (in_maps, T, perms). perms[k] = sorted-order node ids."""
    import ml_dtypes
    x = np.ascontiguousarray(np.asarray(x), dtype=np.float32)
    ei = np.asarray(edge_index)
    src = ei[0].astype(np.int64)
    dst = ei[1].astype(np.int64)
    xpad = np.zeros((N + 1, D), np.float32)
    xpad[:N] = x
    xpad_bf = xpad.astype(ml_dtypes.bfloat16)

    core = dst // NPC
    per_core = []
    T_common = np.ones(NSUP, np.int64)
    for k in range(NC):
        m = core == k
        s_k = src[m]
        d_k = dst[m] - k * NPC
        deg = np.bincount(d_k, minlength=NPC)
        order = np.argsort(-deg, kind="stable")        # old ids, sorted desc
        newpos = np.empty(NPC, np.int64)
        newpos[order] = np.arange(NPC)
        deg_sorted = deg[order]
        # tier count per supertile = max degree in it (= first, sorted)
        maxd = deg_sorted[::SUP][:NSUP]
        Tk = np.ones(NSUP, np.int64)
        Tk[:len(maxd)] = np.maximum(1, maxd)
        T_common = np.maximum(T_common, Tk)
        # rank of each edge within its dst node
        o2 = np.argsort(newpos[d_k], kind="stable")
        s_k = s_k[o2]
        mpos = newpos[d_k[o2]]                         # sorted node pos per edge
        cnt = np.bincount(mpos, minlength=NPAD)
        cum = np.zeros(NPAD + 1, np.int64)
        np.cumsum(cnt, out=cum[1:])
        rank = np.arange(len(mpos)) - cum[mpos]
        per_core.append((s_k, mpos, rank, order))

    # enforce monotone T across each group's supertiles (required by the
    # prefix-fused matmul plan); sorting makes it near-monotone already.
    for (sa, sb) in _groups():
        run = T_common[sa:sb]
        T_common[sa:sb] = np.maximum.accumulate(run[::-1])[::-1]

    T = tuple(int(v) for v in T_common)
    colmap, plan, NBLK = _plan(T)

    s1 = np.eye(128, dtype=ml_dtypes.bfloat16)

    in_maps = []
    for k in range(NC):
        s_k, mpos, rank, order = per_core[k]
        blk = colmap[mpos // SUP, rank]                # block column per edge
        assert (blk >= 0).all()
        p = mpos % SUP                                 # partition per edge
        offs = np.full((128, NBLK), N, np.int64)
        offs[p, blk] = s_k
        xjm = xpad_bf[offs.reshape(-1)].reshape(128, NBLK * D)
        in_maps.append({"xj": xjm, "s1": s1})
    return in_maps, T, [pc[3] for pc in per_core]


def kernel(x, edge_index):
    from concourse import bass_utils

    in_maps, T, perms = _prep_inputs(x, edge_index)
    key = (T, 1)
    if key not in _cache:
        _cache[key] = _build(T, 1)
    nc = _cache[key]

    res = None
    for attempt in range(3):
        try:
            res = bass_utils.run_bass_kernel_spmd(nc, in_maps,
                                                  core_ids=list(range(NC)))
            break
        except Exception:
            if attempt == 2:
                raise
    out = np.empty((N, D), np.float32)
    for k in range(NC):
        y = np.asarray(res.results[k]["y"], np.float32)
        y = y.reshape(128, NSUP, D).transpose(1, 0, 2).reshape(NPAD, D)
        out[k * NPC + perms[k]] = y[:NPC]
    return out
